# revision 14
# baseline (speedup 1.0000x reference)
"""Trainium2 Bass kernel for nn_CriticNetwork (gnn_message_passing).

Math: the reference GNN does mean-aggregation over a complete graph with
self-loops, so every node of an env sees the identical per-env mean.  The
whole network collapses to per-env scalars:

  m_b  = mean over the 16 nodes of obs[b]                      [128]
  p_b  = relu(m_b @ W1 + b1) @ W2 + b2                         [64]
  a_b  = p_b . (Wfc @ (Wattn[:64] + Wattn[64:]))               scalar
  w_b  = sigmoid(leaky_relu(a_b, 0.01))                        scalar
  c_b  = p_b . Wv[:64] + bv                                    scalar
  P_bk = pi[b,k] . Wvy ;  A_bk = act[b,k] . Wvy                (Wvy = Wv[64:72])
  xv[b,j] = c_b + (PS_b + w_b*(AS_b-PS_b) - w_b*(A_bj-P_bj))/16
  out x[b*16+d, j] = xv[b,j]   (independent of d)
  out w[b*16+d, j] = w_b

Sharding: data-parallel over envs, 512 envs per core x 8 cores.

Per-core layout: local env e = 128*g + p (g = group, p = partition), so a
group's per-env scalars live one-per-partition and phase-B tiles hold env
column-blocks g with no cross-partition shuffles.
"""

import numpy as np
from contextlib import ExitStack

import concourse.bass as bass
import concourse.bacc as bacc
import concourse.tile as tile
from concourse import mybir
from concourse.bass_utils import run_bass_kernel_spmd

B, N, A = 4096, 16, 8
D_IN, H1, DP, DZ = 128, 64, 64, 64
NCORES = 8
BC = B // NCORES          # 512 envs per core
RC = BC * N               # 8192 obs rows per core
G = 4                     # env groups per core
GE = BC // G              # 128 envs per group
CW = 272                  # const tile width

F32 = mybir.dt.float32
ALU = mybir.AluOpType
AFT = mybir.ActivationFunctionType


def _build():
    nc = bacc.Bacc("TRN2", target_bir_lowering=False, debug=False)

    obs = nc.dram_tensor("obs", [RC, D_IN], F32, kind="ExternalInput")
    pol = nc.dram_tensor("pol", [RC, A], F32, kind="ExternalInput")
    act = nc.dram_tensor("act", [RC, A], F32, kind="ExternalInput")
    cst = nc.dram_tensor("cst", [128, CW], F32, kind="ExternalInput")
    xo = nc.dram_tensor("xo", [RC, N], F32, kind="ExternalOutput")
    wo = nc.dram_tensor("wo", [RC, N], F32, kind="ExternalOutput")

    with ExitStack() as ctx:
        tc = ctx.enter_context(tile.TileContext(nc))
        consts = ctx.enter_context(tc.tile_pool(name="consts", bufs=1))
        obsp = ctx.enter_context(tc.tile_pool(name="obsp", bufs=4))
        pap = ctx.enter_context(tc.tile_pool(name="pap", bufs=1))
        sb = ctx.enter_context(tc.tile_pool(name="sb", bufs=2))
        sbB = ctx.enter_context(tc.tile_pool(name="sbB", bufs=1))
        pmtp = ctx.enter_context(tc.tile_pool(name="pmtp", bufs=2, space="PSUM"))
        php = ctx.enter_context(tc.tile_pool(name="php", bufs=2, space="PSUM"))
        ppp = ctx.enter_context(tc.tile_pool(name="ppp", bufs=2, space="PSUM"))
        pacp = ctx.enter_context(tc.tile_pool(name="pacp", bufs=1, space="PSUM"))
        pwtp = ctx.enter_context(tc.tile_pool(name="pwtp", bufs=1, space="PSUM"))

        cst_sb = consts.tile([128, CW], F32)
        nc.sync.dma_start(out=cst_sb, in_=cst.ap())
        wvy8_sb = cst_sb[:, 0:8]            # Wvy on all partitions
        w1q_sb = cst_sb[:, 8:72]            # W1 / 16
        w2_sb = cst_sb[0:64, 72:136]
        wac_sb = cst_sb[0:64, 136:138]      # [wa | Wv[:64]]
        b1_sb = cst_sb[0:64, 138:139]
        b2_sb = cst_sb[0:64, 139:140]
        bias2_sb = cst_sb[0:2, 140:141]     # [0, bv]
        id2_sb = cst_sb[0:2, 142:144]       # eye(2)
        id128_sb = cst_sb[:, 144:272]       # eye(128)

        # obs rows for env e=128g+p: 16e..16e+15 -> group g tile [128, 2048]
        obs_v = obs.ap().rearrange("(g p nf) f -> g p (nf f)", p=128, nf=16)

        meanT = consts.tile([128, BC], F32)      # [feat, env]
        wc8 = sbB.tile([128, 8], F32)            # cols 0-3: w_g, 4-7: c_g

        obs_tiles = []
        for g in range(G):
            obs_t = obsp.tile([128, 16 * 128], F32, name="obs_t")
            nc.sync.dma_start(out=obs_t, in_=obs_v[g])
            obs_tiles.append(obs_t)

        # pol/act with interleaved env layout: partition p, block g = env 128g+p
        pa_view = lambda t: t.ap().rearrange("(g p n) a -> p g (n a)", p=128, n=16)
        pol_sb = pap.tile([128, G, N * A], F32)
        nc.sync.dma_start(out=pol_sb, in_=pa_view(pol))
        act_sb = pap.tile([128, G, N * A], F32)
        nc.sync.dma_start(out=act_sb, in_=pa_view(act))

        for g in range(G):
            obs_t = obs_tiles[g]
            # sum over the 16 nodes: one tree level on POOL, rest on DVE
            s1 = sb.tile([128, 1024], F32, name="s1")
            nc.gpsimd.tensor_add(s1, obs_t[:, 0:1024], obs_t[:, 1024:2048])
            s2 = sb.tile([128, 512], F32, name="s2")
            nc.vector.tensor_add(s2, s1[:, 0:512], s1[:, 512:1024])
            s3 = sb.tile([128, 256], F32, name="s3")
            nc.vector.tensor_add(s3, s2[:, 0:256], s2[:, 256:512])
            meanS = sb.tile([128, 128], F32, name="meanS")
            nc.vector.tensor_add(meanS, s3[:, 0:128], s3[:, 128:256])
            pmt = pmtp.tile([128, 128], F32, name="pmt")
            nc.tensor.transpose(pmt, meanS[:], id128_sb)
            nc.vector.tensor_copy(meanT[:, g * GE:(g + 1) * GE], pmt)

            # chain: (sum/16) @ W1 + b1 -> relu -> @W2 + b2 -> [a|c]
            ph = php.tile([64, GE], F32, name="ph")
            nc.tensor.matmul(ph, lhsT=w1q_sb, rhs=meanT[:, g * GE:(g + 1) * GE],
                             start=True, stop=True)
            h_sb = sb.tile([64, GE], F32, name="h_sb")
            nc.scalar.activation(out=h_sb, in_=ph, func=AFT.Relu, bias=b1_sb)
            pp = ppp.tile([64, GE], F32, name="pp")
            nc.tensor.matmul(pp, lhsT=w2_sb, rhs=h_sb, start=True, stop=True)
            p_sb = sb.tile([64, GE], F32, name="p_sb")
            nc.scalar.activation(out=p_sb, in_=pp, func=AFT.Identity, bias=b2_sb)
            pac = pacp.tile([2, GE], F32, name="pac")
            nc.tensor.matmul(pac, lhsT=wac_sb, rhs=p_sb, start=True, stop=True)
            wc = sb.tile([2, GE], F32, name="wc")
            nc.vector.tensor_scalar_add(wc, pac, bias2_sb)
            lr = sb.tile([1, GE], F32, name="lr")
            nc.vector.scalar_tensor_tensor(out=lr, in0=wc[0:1, :], scalar=0.01,
                                           in1=wc[0:1, :], op0=ALU.mult,
                                           op1=ALU.max)
            nc.scalar.activation(out=wc[0:1, :], in_=lr, func=AFT.Sigmoid)
            # per-env scalars onto partitions: [2, 128] -> [128, 2]
            pwt = pwtp.tile([128, 2], F32, name="pwt")
            nc.tensor.transpose(pwt, wc[:], id2_sb)
            wc8_dst = bass.AP(tensor=wc8.tensor, offset=wc8.offset + g,
                              ap=[wc8.ap[0], [4, 2]])
            nc.vector.tensor_copy(wc8_dst, pwt)

        # ---- batched per-node dots: P = pi.Wvy, A = act.Wvy ----
        wvyb = wvy8_sb.unsqueeze(1).unsqueeze(1).broadcast_to([128, G, 16, 8])
        tmP = sbB.tile([128, G, N * A], F32)
        nc.vector.tensor_tensor(out=tmP.rearrange("p g (r a) -> p g r a", a=8),
                                in0=pol_sb.rearrange("p g (r a) -> p g r a", a=8),
                                in1=wvyb, op=ALU.mult)
        tmA = sbB.tile([128, G, N * A], F32)
        nc.vector.tensor_tensor(out=tmA.rearrange("p g (r a) -> p g r a", a=8),
                                in0=act_sb.rearrange("p g (r a) -> p g r a", a=8),
                                in1=wvyb, op=ALU.mult)
        P64 = sbB.tile([128, 64], F32)
        nc.vector.reduce_sum(out=P64,
                             in_=tmP.rearrange("p g (r a) -> p (g r) a", a=8),
                             axis=mybir.AxisListType.X)
        A64 = sbB.tile([128, 64], F32)
        nc.vector.reduce_sum(out=A64,
                             in_=tmA.rearrange("p g (r a) -> p (g r) a", a=8),
                             axis=mybir.AxisListType.X)
        Q64 = sbB.tile([128, 64], F32)
        nc.vector.tensor_sub(Q64, A64, P64)
        PS4 = sbB.tile([128, 4], F32)
        nc.vector.reduce_sum(out=PS4, in_=P64.rearrange("p (i n) -> p i n", n=16),
                             axis=mybir.AxisListType.X)
        AS4 = sbB.tile([128, 4], F32)
        nc.vector.reduce_sum(out=AS4, in_=A64.rearrange("p (i n) -> p i n", n=16),
                             axis=mybir.AxisListType.X)
        QS4 = sbB.tile([128, 4], F32)
        nc.vector.tensor_sub(QS4, AS4, PS4)

        # ---- combine: xv = c + (PS + w*QS)/16 - (w/16)*Q ----
        wT4 = wc8[:, 0:4]
        cT4 = wc8[:, 4:8]
        negw4 = sbB.tile([128, 4], F32)
        nc.scalar.mul(negw4, wT4, -1.0 / N)
        t2 = sbB.tile([128, 4], F32)
        nc.vector.tensor_mul(t2, wT4, QS4)
        t3 = sbB.tile([128, 4], F32)
        nc.vector.tensor_add(t3, t2, PS4)
        base4 = sbB.tile([128, 4], F32)
        nc.vector.scalar_tensor_tensor(out=base4, in0=t3, scalar=1.0 / N,
                                       in1=cT4, op0=ALU.mult, op1=ALU.add)
        nwq = sbB.tile([128, 64], F32)
        nc.vector.tensor_tensor(out=nwq.rearrange("p (i n) -> p i n", n=16),
                                in0=Q64.rearrange("p (i n) -> p i n", n=16),
                                in1=negw4.unsqueeze(2).broadcast_to([128, 4, 16]),
                                op=ALU.mult)
        xv64 = sbB.tile([128, 64], F32)
        nc.vector.tensor_tensor(out=xv64.rearrange("p (i n) -> p i n", n=16),
                                in0=nwq.rearrange("p (i n) -> p i n", n=16),
                                in1=base4.unsqueeze(2).broadcast_to([128, 4, 16]),
                                op=ALU.add)
        w64 = sbB.tile([128, 64], F32)
        nc.vector.tensor_copy(w64.rearrange("p (i n) -> p i n", n=16),
                              wT4.unsqueeze(2).broadcast_to([128, 4, 16]))

        # ---- outputs: env e = 128g+p occupies rows 16e..16e+15 ----
        xo_v = xo.ap().rearrange("(g p d) j -> g p d j", p=128, d=16)
        wo_v = wo.ap().rearrange("(g p d) j -> g p d j", p=128, d=16)
        for g in range(4):
            nc.sync.dma_start(
                out=wo_v[g],
                in_=w64[:, 16 * g:16 * (g + 1)].unsqueeze(1)
                    .broadcast_to([128, 16, 16]))
            nc.scalar.dma_start(
                out=xo_v[g],
                in_=xv64[:, 16 * g:16 * (g + 1)].unsqueeze(1)
                    .broadcast_to([128, 16, 16]))

    nc.compile()
    return nc


_NC_CACHE = {}


def _get_nc():
    if "nc" not in _NC_CACHE:
        _NC_CACHE["nc"] = _build()
    return _NC_CACHE["nc"]


def _make_in_maps(inputs):
    obs = np.ascontiguousarray(np.asarray(inputs["obs"], np.float32))
    pol = np.ascontiguousarray(np.asarray(inputs["policies"], np.float32))
    act = np.ascontiguousarray(np.asarray(inputs["actions"], np.float32))
    W1 = np.asarray(inputs["W1"], np.float32)
    b1 = np.asarray(inputs["b1"], np.float32)
    W2 = np.asarray(inputs["W2"], np.float32)
    b2 = np.asarray(inputs["b2"], np.float32)
    Wfc = np.asarray(inputs["Wfc"], np.float32)
    Wattn = np.asarray(inputs["Wattn"], np.float32)
    Wv = np.asarray(inputs["Wv"], np.float32)
    bv = np.asarray(inputs["bv"], np.float32)

    wa = (Wfc @ (Wattn[:DZ] + Wattn[DZ:]))[:, 0]     # [64]
    wvy = Wv[DP:, 0]                                  # [8]

    cst = np.zeros((128, CW), np.float32)
    cst[:, 0:8] = wvy[None, :]
    cst[:, 8:72] = W1 / 16.0
    cst[0:64, 72:136] = W2
    cst[0:64, 136] = wa
    cst[0:64, 137] = Wv[:DP, 0]
    cst[0:64, 138] = b1
    cst[0:64, 139] = b2
    cst[0, 140] = 0.0
    cst[1, 140] = bv[0]
    cst[0:2, 142:144] = np.eye(2, dtype=np.float32)
    cst[:, 144:272] = np.eye(128, dtype=np.float32)

    in_maps = []
    for c in range(NCORES):
        in_maps.append({
            "obs": obs[c * RC:(c + 1) * RC],
            "pol": pol[c * RC:(c + 1) * RC],
            "act": act[c * RC:(c + 1) * RC],
            "cst": cst,
        })
    return in_maps


# Test-harness knobs (the grader just calls kernel() with defaults).
TRACE = False
TRACE_KWARGS = {}
LAST_RESULT = None


def kernel(**inputs):
    global LAST_RESULT
    nc = _get_nc()
    in_maps = _make_in_maps(inputs)
    res = run_bass_kernel_spmd(nc, in_maps, core_ids=list(range(NCORES)),
                               trace=TRACE, **TRACE_KWARGS)
    LAST_RESULT = res
    x = np.concatenate([r["xo"] for r in res.results], axis=0).reshape(B * N, N, 1)
    w = np.concatenate([r["wo"] for r in res.results], axis=0).reshape(B * N, N, 1)
    return x, w


# revision 17
# speedup vs baseline: 1.3275x; 1.3275x over previous
"""Trainium2 Bass kernel for nn_CriticNetwork (gnn_message_passing).

Math: the reference GNN does mean-aggregation over a complete graph with
self-loops, so every node of an env sees the identical per-env mean.  The
whole network collapses to per-env scalars:

  m_b  = mean over the 16 nodes of obs[b]                      [128]
  p_b  = relu(m_b @ W1 + b1) @ W2 + b2                         [64]
  a_b  = p_b . (Wfc @ (Wattn[:64] + Wattn[64:]))               scalar
  w_b  = sigmoid(leaky_relu(a_b, 0.01))                        scalar
  c_b  = p_b . Wv[:64] + bv                                    scalar
  P_bk = pi[b,k] . Wvy ;  A_bk = act[b,k] . Wvy                (Wvy = Wv[64:72])
  xv[b,j] = c_b + (PS_b + w_b*(AS_b-PS_b) - w_b*(A_bj-P_bj))/16
  out x[b*16+d, j] = xv[b,j]   (independent of d)
  out w[b*16+d, j] = w_b

Sharding: data-parallel over envs, 512 envs per core x 8 cores.

Per-core layout: local env e = 128*g + p (g = group, p = partition), so a
group's per-env scalars live one-per-partition and phase-B tiles hold env
column-blocks g with no cross-partition shuffles.
"""

import numpy as np
from contextlib import ExitStack

import concourse.bass as bass
import concourse.bacc as bacc
import concourse.tile as tile
from concourse import mybir
from concourse.bass_utils import run_bass_kernel_spmd

B, N, A = 4096, 16, 8
D_IN, H1, DP, DZ = 128, 64, 64, 64
NCORES = 8
BC = B // NCORES          # 512 envs per core
RC = BC * N               # 8192 obs rows per core
G = 4                     # env groups per core
GE = BC // G              # 128 envs per group
CW = 272                  # const tile width

F32 = mybir.dt.float32
ALU = mybir.AluOpType
AFT = mybir.ActivationFunctionType


def _build():
    nc = bacc.Bacc("TRN2", target_bir_lowering=False, debug=False)

    obs = nc.dram_tensor("obs", [RC, D_IN], F32, kind="ExternalInput")
    pol = nc.dram_tensor("pol", [RC, A], F32, kind="ExternalInput")
    act = nc.dram_tensor("act", [RC, A], F32, kind="ExternalInput")
    cst = nc.dram_tensor("cst", [128, CW], F32, kind="ExternalInput")
    xo = nc.dram_tensor("xo", [RC, N], F32, kind="ExternalOutput")
    wo = nc.dram_tensor("wo", [RC, N], F32, kind="ExternalOutput")

    with ExitStack() as ctx:
        tc = ctx.enter_context(tile.TileContext(nc))
        consts = ctx.enter_context(tc.tile_pool(name="consts", bufs=1))
        obsp = ctx.enter_context(tc.tile_pool(name="obsp", bufs=4))
        pap = ctx.enter_context(tc.tile_pool(name="pap", bufs=1))
        sb = ctx.enter_context(tc.tile_pool(name="sb", bufs=2))
        sbB = ctx.enter_context(tc.tile_pool(name="sbB", bufs=1))
        pmtp = ctx.enter_context(tc.tile_pool(name="pmtp", bufs=2, space="PSUM"))
        php = ctx.enter_context(tc.tile_pool(name="php", bufs=2, space="PSUM"))
        ppp = ctx.enter_context(tc.tile_pool(name="ppp", bufs=2, space="PSUM"))
        pacp = ctx.enter_context(tc.tile_pool(name="pacp", bufs=1, space="PSUM"))
        pwtp = ctx.enter_context(tc.tile_pool(name="pwtp", bufs=1, space="PSUM"))

        cst_sb = consts.tile([128, CW], F32)
        nc.sync.dma_start(out=cst_sb, in_=cst.ap())
        wvy8_sb = cst_sb[:, 0:8]            # Wvy on all partitions
        w1q_sb = cst_sb[:, 8:72]            # W1 / 16
        w2_sb = cst_sb[0:64, 72:136]
        wac_sb = cst_sb[0:64, 136:138]      # [wa | Wv[:64]]
        b1_sb = cst_sb[0:64, 138:139]
        b2_sb = cst_sb[0:64, 139:140]
        bias2_sb = cst_sb[0:2, 140:141]     # [0, bv]
        id2_sb = cst_sb[0:2, 142:144]       # eye(2)
        id128_sb = cst_sb[:, 144:272]       # eye(128)

        # obs rows for env e=128g+p: 16e..16e+15 -> group g tile [128, 2048]
        obs_v = obs.ap().rearrange("(g p nf) f -> g p (nf f)", p=128, nf=16)

        wc8 = sbB.tile([128, 8], F32)            # cols 0-3: w_g, 4-7: c_g

        # preload the sigmoid ACT table while DMAs stream
        warm = consts.tile([1, 1], F32)
        nc.scalar.activation(out=warm, in_=cst_sb[0:1, 0:1], func=AFT.Sigmoid)

        obs_tiles = []
        for g in range(G):
            obs_t = obsp.tile([128, 16 * 128], F32, name="obs_t")
            nc.sync.dma_start(out=obs_t, in_=obs_v[g])
            obs_tiles.append(obs_t)

        # pol/act with interleaved env layout: partition p, block g = env 128g+p
        pa_view = lambda t: t.ap().rearrange("(g p n) a -> p g (n a)", p=128, n=16)
        pol_sb = pap.tile([128, G, N * A], F32)
        nc.sync.dma_start(out=pol_sb, in_=pa_view(pol))
        act_sb = pap.tile([128, G, N * A], F32)
        nc.sync.dma_start(out=act_sb, in_=pa_view(act))

        for g in range(G):
            obs_t = obs_tiles[g]
            # sum over the 16 nodes: pairwise tree, all on DVE (POOL shares
            # SBUF ports with DVE - running both concurrently slows both)
            s1 = sb.tile([128, 1024], F32, name="s1")
            nc.vector.tensor_add(s1, obs_t[:, 0:1024], obs_t[:, 1024:2048])
            s2 = sb.tile([128, 512], F32, name="s2")
            nc.vector.tensor_add(s2, s1[:, 0:512], s1[:, 512:1024])
            s3 = sb.tile([128, 256], F32, name="s3")
            nc.vector.tensor_add(s3, s2[:, 0:256], s2[:, 256:512])
            meanS = sb.tile([128, 128], F32, name="meanS")
            nc.vector.tensor_add(meanS, s3[:, 0:128], s3[:, 128:256])
            pmt = pmtp.tile([128, 128], F32, name="pmt")
            nc.tensor.transpose(pmt, meanS[:], id128_sb)
            meanT = sb.tile([128, GE], F32, name="meanT")
            nc.scalar.activation(out=meanT, in_=pmt, func=AFT.Copy)

            # chain: (sum/16) @ W1 + b1 -> relu -> @W2 + b2 -> [a|c]
            ph = php.tile([64, GE], F32, name="ph")
            nc.tensor.matmul(ph, lhsT=w1q_sb, rhs=meanT[:],
                             start=True, stop=True)
            h_sb = sb.tile([64, GE], F32, name="h_sb")
            nc.scalar.activation(out=h_sb, in_=ph, func=AFT.Relu, bias=b1_sb)
            pp = ppp.tile([64, GE], F32, name="pp")
            nc.tensor.matmul(pp, lhsT=w2_sb, rhs=h_sb, start=True, stop=True)
            p_sb = sb.tile([64, GE], F32, name="p_sb")
            nc.scalar.activation(out=p_sb, in_=pp, func=AFT.Identity, bias=b2_sb)
            pac = pacp.tile([2, GE], F32, name="pac")
            nc.tensor.matmul(pac, lhsT=wac_sb, rhs=p_sb, start=True, stop=True)
            wc = sb.tile([2, GE], F32, name="wc")
            nc.vector.tensor_scalar_add(wc, pac, bias2_sb)
            lr = sb.tile([1, GE], F32, name="lr")
            nc.vector.scalar_tensor_tensor(out=lr, in0=wc[0:1, :], scalar=0.01,
                                           in1=wc[0:1, :], op0=ALU.mult,
                                           op1=ALU.max)
            nc.scalar.activation(out=wc[0:1, :], in_=lr, func=AFT.Sigmoid)
            # per-env scalars onto partitions: [2, 128] -> [128, 2]
            pwt = pwtp.tile([128, 2], F32, name="pwt")
            nc.tensor.transpose(pwt, wc[:], id2_sb)
            wc8_dst = bass.AP(tensor=wc8.tensor, offset=wc8.offset + g,
                              ap=[wc8.ap[0], [4, 2]])
            nc.vector.tensor_copy(wc8_dst, pwt)

        # ---- batched per-node dots: P = pi.Wvy, A = act.Wvy ----
        wvyb = wvy8_sb.unsqueeze(1).unsqueeze(1).broadcast_to([128, G, 16, 8])
        tmP = sbB.tile([128, G, N * A], F32)
        nc.vector.tensor_tensor(out=tmP.rearrange("p g (r a) -> p g r a", a=8),
                                in0=pol_sb.rearrange("p g (r a) -> p g r a", a=8),
                                in1=wvyb, op=ALU.mult)
        tmA = sbB.tile([128, G, N * A], F32)
        nc.vector.tensor_tensor(out=tmA.rearrange("p g (r a) -> p g r a", a=8),
                                in0=act_sb.rearrange("p g (r a) -> p g r a", a=8),
                                in1=wvyb, op=ALU.mult)
        P64 = sbB.tile([128, 64], F32)
        nc.vector.reduce_sum(out=P64,
                             in_=tmP.rearrange("p g (r a) -> p (g r) a", a=8),
                             axis=mybir.AxisListType.X)
        A64 = sbB.tile([128, 64], F32)
        nc.vector.reduce_sum(out=A64,
                             in_=tmA.rearrange("p g (r a) -> p (g r) a", a=8),
                             axis=mybir.AxisListType.X)
        Q64 = sbB.tile([128, 64], F32)
        nc.vector.tensor_sub(Q64, A64, P64)
        PS4 = sbB.tile([128, 4], F32)
        nc.vector.reduce_sum(out=PS4, in_=P64.rearrange("p (i n) -> p i n", n=16),
                             axis=mybir.AxisListType.X)
        AS4 = sbB.tile([128, 4], F32)
        nc.vector.reduce_sum(out=AS4, in_=A64.rearrange("p (i n) -> p i n", n=16),
                             axis=mybir.AxisListType.X)
        QS4 = sbB.tile([128, 4], F32)
        nc.vector.tensor_sub(QS4, AS4, PS4)

        # ---- combine: xv = c + (PS + w*QS)/16 - (w/16)*Q ----
        wT4 = wc8[:, 0:4]
        cT4 = wc8[:, 4:8]
        negw4 = sbB.tile([128, 4], F32)
        nc.scalar.mul(negw4, wT4, -1.0 / N)
        t2 = sbB.tile([128, 4], F32)
        nc.vector.tensor_mul(t2, wT4, QS4)
        t3 = sbB.tile([128, 4], F32)
        nc.vector.tensor_add(t3, t2, PS4)
        base4 = sbB.tile([128, 4], F32)
        nc.vector.scalar_tensor_tensor(out=base4, in0=t3, scalar=1.0 / N,
                                       in1=cT4, op0=ALU.mult, op1=ALU.add)
        nwq = sbB.tile([128, 64], F32)
        nc.vector.tensor_tensor(out=nwq.rearrange("p (i n) -> p i n", n=16),
                                in0=Q64.rearrange("p (i n) -> p i n", n=16),
                                in1=negw4.unsqueeze(2).broadcast_to([128, 4, 16]),
                                op=ALU.mult)
        xv64 = sbB.tile([128, 64], F32)
        nc.vector.tensor_tensor(out=xv64.rearrange("p (i n) -> p i n", n=16),
                                in0=nwq.rearrange("p (i n) -> p i n", n=16),
                                in1=base4.unsqueeze(2).broadcast_to([128, 4, 16]),
                                op=ALU.add)
        # ---- outputs: env e = 128g+p occupies rows 16e..16e+15 ----
        # materialize full [128, 4*16*16] payloads, then 2 plain fast DMAs
        wbig = sbB.tile([128, G, 16, 16], F32)
        nc.vector.tensor_copy(
            wbig.rearrange("p g a b -> p g (a b)"),
            wT4.unsqueeze(2).broadcast_to([128, 4, 256]))
        xbig = sbB.tile([128, G, 16, 16], F32)
        nc.vector.tensor_copy(
            xbig, xv64.rearrange("p (g j) -> p g j", g=4).unsqueeze(2)
                .broadcast_to([128, 4, 16, 16]))
        xo_v = xo.ap().rearrange("(g p d) j -> p g (d j)", p=128, d=16)
        wo_v = wo.ap().rearrange("(g p d) j -> p g (d j)", p=128, d=16)
        nc.sync.dma_start(out=wo_v, in_=wbig)
        nc.scalar.dma_start(out=xo_v, in_=xbig)

    nc.compile()
    return nc


_NC_CACHE = {}


def _get_nc():
    if "nc" not in _NC_CACHE:
        _NC_CACHE["nc"] = _build()
    return _NC_CACHE["nc"]


def _make_in_maps(inputs):
    obs = np.ascontiguousarray(np.asarray(inputs["obs"], np.float32))
    pol = np.ascontiguousarray(np.asarray(inputs["policies"], np.float32))
    act = np.ascontiguousarray(np.asarray(inputs["actions"], np.float32))
    W1 = np.asarray(inputs["W1"], np.float32)
    b1 = np.asarray(inputs["b1"], np.float32)
    W2 = np.asarray(inputs["W2"], np.float32)
    b2 = np.asarray(inputs["b2"], np.float32)
    Wfc = np.asarray(inputs["Wfc"], np.float32)
    Wattn = np.asarray(inputs["Wattn"], np.float32)
    Wv = np.asarray(inputs["Wv"], np.float32)
    bv = np.asarray(inputs["bv"], np.float32)

    wa = (Wfc @ (Wattn[:DZ] + Wattn[DZ:]))[:, 0]     # [64]
    wvy = Wv[DP:, 0]                                  # [8]

    cst = np.zeros((128, CW), np.float32)
    cst[:, 0:8] = wvy[None, :]
    cst[:, 8:72] = W1 / 16.0
    cst[0:64, 72:136] = W2
    cst[0:64, 136] = wa
    cst[0:64, 137] = Wv[:DP, 0]
    cst[0:64, 138] = b1
    cst[0:64, 139] = b2
    cst[0, 140] = 0.0
    cst[1, 140] = bv[0]
    cst[0:2, 142:144] = np.eye(2, dtype=np.float32)
    cst[:, 144:272] = np.eye(128, dtype=np.float32)

    in_maps = []
    for c in range(NCORES):
        in_maps.append({
            "obs": obs[c * RC:(c + 1) * RC],
            "pol": pol[c * RC:(c + 1) * RC],
            "act": act[c * RC:(c + 1) * RC],
            "cst": cst,
        })
    return in_maps


# Test-harness knobs (the grader just calls kernel() with defaults).
TRACE = False
TRACE_KWARGS = {}
LAST_RESULT = None


def kernel(**inputs):
    global LAST_RESULT
    nc = _get_nc()
    in_maps = _make_in_maps(inputs)
    res = run_bass_kernel_spmd(nc, in_maps, core_ids=list(range(NCORES)),
                               trace=TRACE, **TRACE_KWARGS)
    LAST_RESULT = res
    x = np.concatenate([r["xo"] for r in res.results], axis=0).reshape(B * N, N, 1)
    w = np.concatenate([r["wo"] for r in res.results], axis=0).reshape(B * N, N, 1)
    return x, w


# revision 24
# speedup vs baseline: 1.4022x; 1.0563x over previous
"""Trainium2 Bass kernel for nn_CriticNetwork (gnn_message_passing).

Math: the reference GNN does mean-aggregation over a complete graph with
self-loops, so every node of an env sees the identical per-env mean.  The
whole network collapses to per-env scalars:

  m_b  = mean over the 16 nodes of obs[b]                      [128]
  p_b  = relu(m_b @ W1 + b1) @ W2 + b2                         [64]
  a_b  = p_b . (Wfc @ (Wattn[:64] + Wattn[64:]))               scalar
  w_b  = sigmoid(leaky_relu(a_b, 0.01))                        scalar
  c_b  = p_b . Wv[:64] + bv                                    scalar
  P_bk = pi[b,k] . Wvy ;  A_bk = act[b,k] . Wvy                (Wvy = Wv[64:72])
  xv[b,j] = c_b + (PS_b + w_b*(AS_b-PS_b) - w_b*(A_bj-P_bj))/16
  out x[b*16+d, j] = xv[b,j]   (independent of d)
  out w[b*16+d, j] = w_b

Sharding: data-parallel over envs, 512 envs per core x 8 cores.

Per-core layout: local env e = 128*g + p (g = group, p = partition), so a
group's per-env scalars live one-per-partition and phase-B tiles hold env
column-blocks g with no cross-partition shuffles.
"""

import numpy as np
from contextlib import ExitStack

import concourse.bass as bass
import concourse.bacc as bacc
import concourse.tile as tile
from concourse import mybir
from concourse.bass_utils import run_bass_kernel_spmd

B, N, A = 4096, 16, 8
D_IN, H1, DP, DZ = 128, 64, 64, 64
NCORES = 8
BC = B // NCORES          # 512 envs per core
RC = BC * N               # 8192 obs rows per core
G = 4                     # env groups per core
GE = BC // G              # 128 envs per group
CW = 272                  # const tile width

F32 = mybir.dt.float32
ALU = mybir.AluOpType
AFT = mybir.ActivationFunctionType


def _build():
    nc = bacc.Bacc("TRN2", target_bir_lowering=False, debug=False)

    obs = nc.dram_tensor("obs", [RC, D_IN], F32, kind="ExternalInput")
    pol = nc.dram_tensor("pol", [RC, A], F32, kind="ExternalInput")
    act = nc.dram_tensor("act", [RC, A], F32, kind="ExternalInput")
    cst = nc.dram_tensor("cst", [128, CW], F32, kind="ExternalInput")
    xo = nc.dram_tensor("xo", [RC, N], F32, kind="ExternalOutput")
    wo = nc.dram_tensor("wo", [RC, N], F32, kind="ExternalOutput")

    with ExitStack() as ctx:
        tc = ctx.enter_context(tile.TileContext(nc))
        consts = ctx.enter_context(tc.tile_pool(name="consts", bufs=1))
        obsp = ctx.enter_context(tc.tile_pool(name="obsp", bufs=4))
        pap = ctx.enter_context(tc.tile_pool(name="pap", bufs=1))
        sb = ctx.enter_context(tc.tile_pool(name="sb", bufs=2))
        sbB = ctx.enter_context(tc.tile_pool(name="sbB", bufs=1))
        pmtp = ctx.enter_context(tc.tile_pool(name="pmtp", bufs=2, space="PSUM"))
        php = ctx.enter_context(tc.tile_pool(name="php", bufs=2, space="PSUM"))
        ppp = ctx.enter_context(tc.tile_pool(name="ppp", bufs=2, space="PSUM"))
        pacp = ctx.enter_context(tc.tile_pool(name="pacp", bufs=1, space="PSUM"))
        pwtp = ctx.enter_context(tc.tile_pool(name="pwtp", bufs=1, space="PSUM"))

        cst_sb = consts.tile([128, CW], F32)
        nc.sync.dma_start(out=cst_sb, in_=cst.ap())
        wvy8_sb = cst_sb[:, 0:8]            # Wvy on all partitions
        w1q_sb = cst_sb[:, 8:72]            # W1 / 16
        wq_sb = cst_sb[0:64, 72:74]         # W2 @ [wa | Wv[:64]]
        b1_sb = cst_sb[0:64, 138:139]
        biasq_sb = cst_sb[0:2, 140:141]     # [b2.wa, b2.Wv64 + bv]
        id2_sb = cst_sb[0:2, 142:144]       # eye(2)
        id128_sb = cst_sb[:, 144:272]       # eye(128)

        # obs rows for env e=128g+p: 16e..16e+15 -> group g tile [128, 2048]
        obs_v = obs.ap().rearrange("(g p nf) f -> g p (nf f)", p=128, nf=16)

        wc8 = sbB.tile([128, 8], F32)            # cols 0-3: w_g, 4-7: c_g

        # preload the sigmoid ACT table while DMAs stream
        warm = consts.tile([1, 1], F32)
        nc.scalar.activation(out=warm, in_=cst_sb[0:1, 0:1], func=AFT.Sigmoid)

        obs_tiles = []
        for g in range(G):
            obs_t = obsp.tile([128, 16 * 128], F32, name="obs_t")
            # alternate HWDGE rings so transfers overlap across both queues
            eng = nc.sync if g % 2 == 0 else nc.scalar
            eng.dma_start(out=obs_t, in_=obs_v[g])
            obs_tiles.append(obs_t)

        # pol/act with interleaved env layout: partition p, block g = env 128g+p
        pa_view = lambda t: t.ap().rearrange("(g p n) a -> p g (n a)", p=128, n=16)
        pol_sb = pap.tile([128, G, N * A], F32)
        nc.sync.dma_start(out=pol_sb, in_=pa_view(pol))
        act_sb = pap.tile([128, G, N * A], F32)
        nc.sync.dma_start(out=act_sb, in_=pa_view(act))

        last_tree_inst = None
        for g in range(G):
            obs_t = obs_tiles[g]
            # sum over the 16 nodes: pairwise tree, all on DVE (POOL shares
            # SBUF ports with DVE - running both concurrently slows both)
            s1 = sb.tile([128, 1024], F32, name="s1")
            nc.vector.tensor_add(s1, obs_t[:, 0:1024], obs_t[:, 1024:2048])
            s2 = sb.tile([128, 512], F32, name="s2")
            nc.vector.tensor_add(s2, s1[:, 0:512], s1[:, 512:1024])
            s3 = sb.tile([128, 256], F32, name="s3")
            nc.vector.tensor_add(s3, s2[:, 0:256], s2[:, 256:512])
            meanS = sb.tile([128, 128], F32, name="meanS")
            last_tree_inst = nc.vector.tensor_add(meanS, s3[:, 0:128],
                                                  s3[:, 128:256])
            pmt = pmtp.tile([128, 128], F32, name="pmt")
            nc.tensor.transpose(pmt, meanS[:], id128_sb)
            meanT = sb.tile([128, GE], F32, name="meanT")
            nc.scalar.activation(out=meanT, in_=pmt, func=AFT.Copy)

            # chain: (sum/16) @ W1 + b1 -> relu -> @(W2@Wac) + biasq -> [a|c]
            ph = php.tile([64, GE], F32, name="ph")
            nc.tensor.matmul(ph, lhsT=w1q_sb, rhs=meanT[:],
                             start=True, stop=True)
            h_sb = sb.tile([64, GE], F32, name="h_sb")
            nc.scalar.activation(out=h_sb, in_=ph, func=AFT.Relu, bias=b1_sb)
            pac = pacp.tile([2, GE], F32, name="pac")
            nc.tensor.matmul(pac, lhsT=wq_sb, rhs=h_sb, start=True, stop=True)
            wc = sb.tile([2, GE], F32, name="wc")
            nc.scalar.activation(out=wc, in_=pac, func=AFT.Identity,
                                 bias=biasq_sb)
            lr = sb.tile([1, GE], F32, name="lr")
            nc.vector.scalar_tensor_tensor(out=lr, in0=wc[0:1, :], scalar=0.01,
                                           in1=wc[0:1, :], op0=ALU.mult,
                                           op1=ALU.max)
            nc.scalar.activation(out=wc[0:1, :], in_=lr, func=AFT.Sigmoid)
            # per-env scalars onto partitions: [2, 128] -> [128, 2]
            pwt = pwtp.tile([128, 2], F32, name="pwt")
            nc.tensor.transpose(pwt, wc[:], id2_sb)
            wc8_dst = bass.AP(tensor=wc8.tensor, offset=wc8.offset + g,
                              ap=[wc8.ap[0], [4, 2]])
            nc.vector.tensor_copy(wc8_dst, pwt)

        # ---- batched per-node dots: P = pi.Wvy, A = act.Wvy ----
        # Keep these off DVE's critical window: order them after the last
        # mean-tree op so group 3's tree isn't interleaved with them.
        wvyb = wvy8_sb.unsqueeze(1).unsqueeze(1).broadcast_to([128, G, 16, 8])
        tmP = sbB.tile([128, G, N * A], F32)
        i_tmP = nc.vector.tensor_tensor(
            out=tmP.rearrange("p g (r a) -> p g r a", a=8),
            in0=pol_sb.rearrange("p g (r a) -> p g r a", a=8),
            in1=wvyb, op=ALU.mult)
        tmA = sbB.tile([128, G, N * A], F32)
        i_tmA = nc.vector.tensor_tensor(
            out=tmA.rearrange("p g (r a) -> p g r a", a=8),
            in0=act_sb.rearrange("p g (r a) -> p g r a", a=8),
            in1=wvyb, op=ALU.mult)
        if last_tree_inst is not None:
            tile.add_dep_helper(i_tmP.ins, last_tree_inst.ins, sync=False,
                                reason="keep DVE free for the last mean tree")
            tile.add_dep_helper(i_tmA.ins, last_tree_inst.ins, sync=False,
                                reason="keep DVE free for the last mean tree")
        P64 = sbB.tile([128, 64], F32)
        nc.vector.reduce_sum(out=P64,
                             in_=tmP.rearrange("p g (r a) -> p (g r) a", a=8),
                             axis=mybir.AxisListType.X)
        A64 = sbB.tile([128, 64], F32)
        nc.vector.reduce_sum(out=A64,
                             in_=tmA.rearrange("p g (r a) -> p (g r) a", a=8),
                             axis=mybir.AxisListType.X)
        Q64 = sbB.tile([128, 64], F32)
        nc.vector.tensor_sub(Q64, A64, P64)
        PS4 = sbB.tile([128, 4], F32)
        nc.vector.reduce_sum(out=PS4, in_=P64.rearrange("p (i n) -> p i n", n=16),
                             axis=mybir.AxisListType.X)
        AS4 = sbB.tile([128, 4], F32)
        nc.vector.reduce_sum(out=AS4, in_=A64.rearrange("p (i n) -> p i n", n=16),
                             axis=mybir.AxisListType.X)
        QS4 = sbB.tile([128, 4], F32)
        nc.vector.tensor_sub(QS4, AS4, PS4)

        # ---- combine: xv = c + (PS + w*QS)/16 - (w/16)*Q ----
        wT4 = wc8[:, 0:4]
        cT4 = wc8[:, 4:8]
        negw4 = sbB.tile([128, 4], F32)
        nc.scalar.mul(negw4, wT4, -1.0 / N)
        t2 = sbB.tile([128, 4], F32)
        nc.vector.tensor_mul(t2, wT4, QS4)
        t3 = sbB.tile([128, 4], F32)
        nc.vector.tensor_add(t3, t2, PS4)
        base4 = sbB.tile([128, 4], F32)
        nc.vector.scalar_tensor_tensor(out=base4, in0=t3, scalar=1.0 / N,
                                       in1=cT4, op0=ALU.mult, op1=ALU.add)
        nwq = sbB.tile([128, 64], F32)
        nc.vector.tensor_tensor(out=nwq.rearrange("p (i n) -> p i n", n=16),
                                in0=Q64.rearrange("p (i n) -> p i n", n=16),
                                in1=negw4.unsqueeze(2).broadcast_to([128, 4, 16]),
                                op=ALU.mult)
        xv64 = sbB.tile([128, 64], F32)
        nc.vector.tensor_tensor(out=xv64.rearrange("p (i n) -> p i n", n=16),
                                in0=nwq.rearrange("p (i n) -> p i n", n=16),
                                in1=base4.unsqueeze(2).broadcast_to([128, 4, 16]),
                                op=ALU.add)
        # ---- outputs: env e = 128g+p occupies rows 16e..16e+15 ----
        # materialize full [128, 4*16*16] payloads, then 2 plain fast DMAs
        wbig = sbB.tile([128, G, 16, 16], F32)
        nc.vector.tensor_copy(
            wbig.rearrange("p g a b -> p g (a b)"),
            wT4.unsqueeze(2).broadcast_to([128, 4, 256]))
        xbig = sbB.tile([128, G, 16, 16], F32)
        nc.vector.tensor_copy(
            xbig, xv64.rearrange("p (g j) -> p g j", g=4).unsqueeze(2)
                .broadcast_to([128, 4, 16, 16]))
        xo_v = xo.ap().rearrange("(g p d) j -> p g (d j)", p=128, d=16)
        wo_v = wo.ap().rearrange("(g p d) j -> p g (d j)", p=128, d=16)
        # split across partition halves and both HWDGE rings so the final
        # transfers overlap
        nc.sync.dma_start(out=wo_v[0:64], in_=wbig[0:64])
        nc.scalar.dma_start(out=wo_v[64:128], in_=wbig[64:128])
        nc.sync.dma_start(out=xo_v[0:64], in_=xbig[0:64])
        nc.scalar.dma_start(out=xo_v[64:128], in_=xbig[64:128])

    nc.compile()
    return nc


_NC_CACHE = {}


def _get_nc():
    if "nc" not in _NC_CACHE:
        _NC_CACHE["nc"] = _build()
    return _NC_CACHE["nc"]


def _make_in_maps(inputs):
    obs = np.ascontiguousarray(np.asarray(inputs["obs"], np.float32))
    pol = np.ascontiguousarray(np.asarray(inputs["policies"], np.float32))
    act = np.ascontiguousarray(np.asarray(inputs["actions"], np.float32))
    W1 = np.asarray(inputs["W1"], np.float32)
    b1 = np.asarray(inputs["b1"], np.float32)
    W2 = np.asarray(inputs["W2"], np.float32)
    b2 = np.asarray(inputs["b2"], np.float32)
    Wfc = np.asarray(inputs["Wfc"], np.float32)
    Wattn = np.asarray(inputs["Wattn"], np.float32)
    Wv = np.asarray(inputs["Wv"], np.float32)
    bv = np.asarray(inputs["bv"], np.float32)

    wa = (Wfc @ (Wattn[:DZ] + Wattn[DZ:]))[:, 0]     # [64]
    wvy = Wv[DP:, 0]                                  # [8]

    wv64 = Wv[:DP, 0]
    cst = np.zeros((128, CW), np.float32)
    cst[:, 0:8] = wvy[None, :]
    cst[:, 8:72] = W1 / 16.0
    cst[0:64, 72] = W2 @ wa                  # Wq col 0
    cst[0:64, 73] = W2 @ wv64                # Wq col 1
    cst[0:64, 138] = b1
    cst[0, 140] = float(b2 @ wa)             # biasq
    cst[1, 140] = float(b2 @ wv64 + bv[0])
    cst[0:2, 142:144] = np.eye(2, dtype=np.float32)
    cst[:, 144:272] = np.eye(128, dtype=np.float32)

    in_maps = []
    for c in range(NCORES):
        in_maps.append({
            "obs": obs[c * RC:(c + 1) * RC],
            "pol": pol[c * RC:(c + 1) * RC],
            "act": act[c * RC:(c + 1) * RC],
            "cst": cst,
        })
    return in_maps


# Test-harness knobs (the grader just calls kernel() with defaults).
TRACE = False
TRACE_KWARGS = {}
LAST_RESULT = None


def kernel(**inputs):
    global LAST_RESULT
    nc = _get_nc()
    in_maps = _make_in_maps(inputs)
    res = run_bass_kernel_spmd(nc, in_maps, core_ids=list(range(NCORES)),
                               trace=TRACE, **TRACE_KWARGS)
    LAST_RESULT = res
    x = np.concatenate([r["xo"] for r in res.results], axis=0).reshape(B * N, N, 1)
    w = np.concatenate([r["wo"] for r in res.results], axis=0).reshape(B * N, N, 1)
    return x, w


# revision 26
# speedup vs baseline: 1.4894x; 1.0621x over previous
"""Trainium2 Bass kernel for nn_CriticNetwork (gnn_message_passing).

Math: the reference GNN does mean-aggregation over a complete graph with
self-loops, so every node of an env sees the identical per-env mean.  The
whole network collapses to per-env scalars:

  m_b  = mean over the 16 nodes of obs[b]                      [128]
  p_b  = relu(m_b @ W1 + b1) @ W2 + b2                         [64]
  a_b  = p_b . (Wfc @ (Wattn[:64] + Wattn[64:]))               scalar
  w_b  = sigmoid(leaky_relu(a_b, 0.01))                        scalar
  c_b  = p_b . Wv[:64] + bv                                    scalar
  P_bk = pi[b,k] . Wvy ;  A_bk = act[b,k] . Wvy                (Wvy = Wv[64:72])
  xv[b,j] = c_b + (PS_b + w_b*(AS_b-PS_b) - w_b*(A_bj-P_bj))/16
  out x[b*16+d, j] = xv[b,j]   (independent of d)
  out w[b*16+d, j] = w_b

Sharding: data-parallel over envs, 512 envs per core x 8 cores.

Per-core layout: local env e = 128*g + p (g = group, p = partition), so a
group's per-env scalars live one-per-partition and phase-B tiles hold env
column-blocks g with no cross-partition shuffles.
"""

import numpy as np
from contextlib import ExitStack

import concourse.bass as bass
import concourse.bacc as bacc
import concourse.tile as tile
from concourse import mybir
from concourse.bass_utils import run_bass_kernel_spmd

B, N, A = 4096, 16, 8
D_IN, H1, DP, DZ = 128, 64, 64, 64
NCORES = 8
BC = B // NCORES          # 512 envs per core
RC = BC * N               # 8192 obs rows per core
G = 4                     # env groups per core
GE = BC // G              # 128 envs per group
CW = 272                  # const tile width

F32 = mybir.dt.float32
ALU = mybir.AluOpType
AFT = mybir.ActivationFunctionType


def _build():
    nc = bacc.Bacc("TRN2", target_bir_lowering=False, debug=False)

    obs = nc.dram_tensor("obs", [RC, D_IN], F32, kind="ExternalInput")
    pol = nc.dram_tensor("pol", [RC, A], F32, kind="ExternalInput")
    act = nc.dram_tensor("act", [RC, A], F32, kind="ExternalInput")
    cst = nc.dram_tensor("cst", [128, CW], F32, kind="ExternalInput")
    xo = nc.dram_tensor("xo", [RC, N], F32, kind="ExternalOutput")
    wo = nc.dram_tensor("wo", [RC, N], F32, kind="ExternalOutput")

    with ExitStack() as ctx:
        tc = ctx.enter_context(tile.TileContext(nc))
        consts = ctx.enter_context(tc.tile_pool(name="consts", bufs=1))
        obsp = ctx.enter_context(tc.tile_pool(name="obsp", bufs=4))
        pap = ctx.enter_context(tc.tile_pool(name="pap", bufs=1))
        sb = ctx.enter_context(tc.tile_pool(name="sb", bufs=2))
        sbB = ctx.enter_context(tc.tile_pool(name="sbB", bufs=1))
        pmtp = ctx.enter_context(tc.tile_pool(name="pmtp", bufs=2, space="PSUM"))
        php = ctx.enter_context(tc.tile_pool(name="php", bufs=2, space="PSUM"))
        ppp = ctx.enter_context(tc.tile_pool(name="ppp", bufs=2, space="PSUM"))
        pacp = ctx.enter_context(tc.tile_pool(name="pacp", bufs=1, space="PSUM"))
        pwtp = ctx.enter_context(tc.tile_pool(name="pwtp", bufs=1, space="PSUM"))

        # cst on the scalar ring so the sync ring starts obs immediately
        cst_sb = consts.tile([128, CW], F32)
        nc.scalar.dma_start(out=cst_sb, in_=cst.ap())
        wvy8_sb = cst_sb[:, 0:8]            # Wvy on all partitions
        w1q_sb = cst_sb[:, 8:72]            # W1 / 16
        wq_sb = cst_sb[0:64, 72:74]         # W2 @ [wa | Wv[:64]]
        b1_sb = cst_sb[0:64, 138:139]
        biasq_sb = cst_sb[0:2, 140:141]     # [b2.wa, b2.Wv64 + bv]
        id2_sb = cst_sb[0:2, 142:144]       # eye(2)
        id128_sb = cst_sb[:, 144:272]       # eye(128)

        # obs rows for env e=128g+p: 16e..16e+15 -> group g tile [128, 2048]
        obs_v = obs.ap().rearrange("(g p nf) f -> g p (nf f)", p=128, nf=16)

        wc8 = sbB.tile([128, 8], F32)            # cols 0-3: w_g, 4-7: c_g

        # preload the sigmoid ACT table while DMAs stream
        warm = consts.tile([1, 1], F32)
        nc.scalar.activation(out=warm, in_=cst_sb[0:1, 0:1], func=AFT.Sigmoid)

        obs_tiles = []
        for g in range(G):
            obs_t = obsp.tile([128, 16 * 128], F32, name="obs_t")
            # all obs on ONE ring: per-ring transfers run serially at near
            # full HBM bandwidth, so group g's data lands ~3us apart and the
            # mean trees/chains pipeline behind the loads. (Splitting across
            # both rings makes every transfer finish late together.)
            nc.sync.dma_start(out=obs_t, in_=obs_v[g])
            obs_tiles.append(obs_t)

        # pol/act with interleaved env layout: partition p, block g = env 128g+p
        pa_view = lambda t: t.ap().rearrange("(g p n) a -> p g (n a)", p=128, n=16)
        pol_sb = pap.tile([128, G, N * A], F32)
        nc.sync.dma_start(out=pol_sb, in_=pa_view(pol))
        act_sb = pap.tile([128, G, N * A], F32)
        nc.sync.dma_start(out=act_sb, in_=pa_view(act))

        last_tree_inst = None
        for g in range(G):
            obs_t = obs_tiles[g]
            # sum over the 16 nodes: pairwise tree, all on DVE (POOL shares
            # SBUF ports with DVE - running both concurrently slows both)
            s1 = sb.tile([128, 1024], F32, name="s1")
            nc.vector.tensor_add(s1, obs_t[:, 0:1024], obs_t[:, 1024:2048])
            s2 = sb.tile([128, 512], F32, name="s2")
            nc.vector.tensor_add(s2, s1[:, 0:512], s1[:, 512:1024])
            s3 = sb.tile([128, 256], F32, name="s3")
            nc.vector.tensor_add(s3, s2[:, 0:256], s2[:, 256:512])
            meanS = sb.tile([128, 128], F32, name="meanS")
            last_tree_inst = nc.vector.tensor_add(meanS, s3[:, 0:128],
                                                  s3[:, 128:256])
            pmt = pmtp.tile([128, 128], F32, name="pmt")
            nc.tensor.transpose(pmt, meanS[:], id128_sb)
            meanT = sb.tile([128, GE], F32, name="meanT")
            nc.scalar.activation(out=meanT, in_=pmt, func=AFT.Copy)

            # chain: (sum/16) @ W1 + b1 -> relu -> @(W2@Wac) + biasq -> [a|c]
            ph = php.tile([64, GE], F32, name="ph")
            nc.tensor.matmul(ph, lhsT=w1q_sb, rhs=meanT[:],
                             start=True, stop=True)
            h_sb = sb.tile([64, GE], F32, name="h_sb")
            nc.scalar.activation(out=h_sb, in_=ph, func=AFT.Relu, bias=b1_sb)
            pac = pacp.tile([2, GE], F32, name="pac")
            nc.tensor.matmul(pac, lhsT=wq_sb, rhs=h_sb, start=True, stop=True)
            wc = sb.tile([2, GE], F32, name="wc")
            nc.scalar.activation(out=wc, in_=pac, func=AFT.Identity,
                                 bias=biasq_sb)
            lr = sb.tile([1, GE], F32, name="lr")
            nc.vector.scalar_tensor_tensor(out=lr, in0=wc[0:1, :], scalar=0.01,
                                           in1=wc[0:1, :], op0=ALU.mult,
                                           op1=ALU.max)
            nc.scalar.activation(out=wc[0:1, :], in_=lr, func=AFT.Sigmoid)
            # per-env scalars onto partitions: [2, 128] -> [128, 2]
            pwt = pwtp.tile([128, 2], F32, name="pwt")
            nc.tensor.transpose(pwt, wc[:], id2_sb)
            wc8_dst = bass.AP(tensor=wc8.tensor, offset=wc8.offset + g,
                              ap=[wc8.ap[0], [4, 2]])
            nc.vector.tensor_copy(wc8_dst, pwt)

        # ---- batched per-node dots: P = pi.Wvy, A = act.Wvy ----
        # Keep these off DVE's critical window: order them after the last
        # mean-tree op so group 3's tree isn't interleaved with them.
        wvyb = wvy8_sb.unsqueeze(1).unsqueeze(1).broadcast_to([128, G, 16, 8])
        tmP = sbB.tile([128, G, N * A], F32)
        i_tmP = nc.vector.tensor_tensor(
            out=tmP.rearrange("p g (r a) -> p g r a", a=8),
            in0=pol_sb.rearrange("p g (r a) -> p g r a", a=8),
            in1=wvyb, op=ALU.mult)
        tmA = sbB.tile([128, G, N * A], F32)
        i_tmA = nc.vector.tensor_tensor(
            out=tmA.rearrange("p g (r a) -> p g r a", a=8),
            in0=act_sb.rearrange("p g (r a) -> p g r a", a=8),
            in1=wvyb, op=ALU.mult)
        if last_tree_inst is not None:
            tile.add_dep_helper(i_tmP.ins, last_tree_inst.ins, sync=False,
                                reason="keep DVE free for the last mean tree")
            tile.add_dep_helper(i_tmA.ins, last_tree_inst.ins, sync=False,
                                reason="keep DVE free for the last mean tree")
        P64 = sbB.tile([128, 64], F32)
        nc.vector.reduce_sum(out=P64,
                             in_=tmP.rearrange("p g (r a) -> p (g r) a", a=8),
                             axis=mybir.AxisListType.X)
        A64 = sbB.tile([128, 64], F32)
        nc.vector.reduce_sum(out=A64,
                             in_=tmA.rearrange("p g (r a) -> p (g r) a", a=8),
                             axis=mybir.AxisListType.X)
        Q64 = sbB.tile([128, 64], F32)
        nc.vector.tensor_sub(Q64, A64, P64)
        PS4 = sbB.tile([128, 4], F32)
        nc.vector.reduce_sum(out=PS4, in_=P64.rearrange("p (i n) -> p i n", n=16),
                             axis=mybir.AxisListType.X)
        AS4 = sbB.tile([128, 4], F32)
        nc.vector.reduce_sum(out=AS4, in_=A64.rearrange("p (i n) -> p i n", n=16),
                             axis=mybir.AxisListType.X)
        QS4 = sbB.tile([128, 4], F32)
        nc.vector.tensor_sub(QS4, AS4, PS4)

        # ---- combine: xv = c + (PS + w*QS)/16 - (w/16)*Q ----
        wT4 = wc8[:, 0:4]
        cT4 = wc8[:, 4:8]
        negw4 = sbB.tile([128, 4], F32)
        nc.scalar.mul(negw4, wT4, -1.0 / N)
        t2 = sbB.tile([128, 4], F32)
        nc.vector.tensor_mul(t2, wT4, QS4)
        t3 = sbB.tile([128, 4], F32)
        nc.vector.tensor_add(t3, t2, PS4)
        base4 = sbB.tile([128, 4], F32)
        nc.vector.scalar_tensor_tensor(out=base4, in0=t3, scalar=1.0 / N,
                                       in1=cT4, op0=ALU.mult, op1=ALU.add)
        nwq = sbB.tile([128, 64], F32)
        nc.vector.tensor_tensor(out=nwq.rearrange("p (i n) -> p i n", n=16),
                                in0=Q64.rearrange("p (i n) -> p i n", n=16),
                                in1=negw4.unsqueeze(2).broadcast_to([128, 4, 16]),
                                op=ALU.mult)
        xv64 = sbB.tile([128, 64], F32)
        nc.vector.tensor_tensor(out=xv64.rearrange("p (i n) -> p i n", n=16),
                                in0=nwq.rearrange("p (i n) -> p i n", n=16),
                                in1=base4.unsqueeze(2).broadcast_to([128, 4, 16]),
                                op=ALU.add)
        # ---- outputs: env e = 128g+p occupies rows 16e..16e+15 ----
        # materialize full [128, 4*16*16] payloads, then 2 plain fast DMAs
        wbig = sbB.tile([128, G, 16, 16], F32)
        nc.vector.tensor_copy(
            wbig.rearrange("p g a b -> p g (a b)"),
            wT4.unsqueeze(2).broadcast_to([128, 4, 256]))
        xbig = sbB.tile([128, G, 16, 16], F32)
        nc.vector.tensor_copy(
            xbig, xv64.rearrange("p (g j) -> p g j", g=4).unsqueeze(2)
                .broadcast_to([128, 4, 16, 16]))
        xo_v = xo.ap().rearrange("(g p d) j -> p g (d j)", p=128, d=16)
        wo_v = wo.ap().rearrange("(g p d) j -> p g (d j)", p=128, d=16)
        # split across partition halves and both HWDGE rings so the final
        # transfers overlap
        nc.sync.dma_start(out=wo_v[0:64], in_=wbig[0:64])
        nc.scalar.dma_start(out=wo_v[64:128], in_=wbig[64:128])
        nc.sync.dma_start(out=xo_v[0:64], in_=xbig[0:64])
        nc.scalar.dma_start(out=xo_v[64:128], in_=xbig[64:128])

    nc.compile()
    return nc


_NC_CACHE = {}


def _get_nc():
    if "nc" not in _NC_CACHE:
        _NC_CACHE["nc"] = _build()
    return _NC_CACHE["nc"]


def _make_in_maps(inputs):
    obs = np.ascontiguousarray(np.asarray(inputs["obs"], np.float32))
    pol = np.ascontiguousarray(np.asarray(inputs["policies"], np.float32))
    act = np.ascontiguousarray(np.asarray(inputs["actions"], np.float32))
    W1 = np.asarray(inputs["W1"], np.float32)
    b1 = np.asarray(inputs["b1"], np.float32)
    W2 = np.asarray(inputs["W2"], np.float32)
    b2 = np.asarray(inputs["b2"], np.float32)
    Wfc = np.asarray(inputs["Wfc"], np.float32)
    Wattn = np.asarray(inputs["Wattn"], np.float32)
    Wv = np.asarray(inputs["Wv"], np.float32)
    bv = np.asarray(inputs["bv"], np.float32)

    wa = (Wfc @ (Wattn[:DZ] + Wattn[DZ:]))[:, 0]     # [64]
    wvy = Wv[DP:, 0]                                  # [8]

    wv64 = Wv[:DP, 0]
    cst = np.zeros((128, CW), np.float32)
    cst[:, 0:8] = wvy[None, :]
    cst[:, 8:72] = W1 / 16.0
    cst[0:64, 72] = W2 @ wa                  # Wq col 0
    cst[0:64, 73] = W2 @ wv64                # Wq col 1
    cst[0:64, 138] = b1
    cst[0, 140] = float(b2 @ wa)             # biasq
    cst[1, 140] = float(b2 @ wv64 + bv[0])
    cst[0:2, 142:144] = np.eye(2, dtype=np.float32)
    cst[:, 144:272] = np.eye(128, dtype=np.float32)

    in_maps = []
    for c in range(NCORES):
        in_maps.append({
            "obs": obs[c * RC:(c + 1) * RC],
            "pol": pol[c * RC:(c + 1) * RC],
            "act": act[c * RC:(c + 1) * RC],
            "cst": cst,
        })
    return in_maps


# Test-harness knobs (the grader just calls kernel() with defaults).
TRACE = False
TRACE_KWARGS = {}
LAST_RESULT = None


def kernel(**inputs):
    global LAST_RESULT
    nc = _get_nc()
    in_maps = _make_in_maps(inputs)
    res = run_bass_kernel_spmd(nc, in_maps, core_ids=list(range(NCORES)),
                               trace=TRACE, **TRACE_KWARGS)
    LAST_RESULT = res
    x = np.concatenate([r["xo"] for r in res.results], axis=0).reshape(B * N, N, 1)
    w = np.concatenate([r["wo"] for r in res.results], axis=0).reshape(B * N, N, 1)
    return x, w
